# revision 18
# baseline (speedup 1.0000x reference)
"""GAT (2-layer, PyG-style) on 8 Trainium2 NeuronCores via Bass/Tile.

Strategy (dst-major graph-parallel, gather-descriptor-minimal):
  - Nodes partitioned across 8 cores by dst id (6250 each); per core, own
    dsts are degree-sorted into 49 blocks of 128.  Block j is a
    [128 dst-partitions x S_j slots] grid; slot (d, s) = s-th in-edge of
    block-dst d.  Self-loops are NOT in the grid (computed inline from the
    own-node matmul), so S_j = max plain in-degree -> ~2.5% slot padding.
  - Per-edge source data is fetched with ONE dma_gather per chunk of blocks
    from a DRAM table addressed by PAIRS of rows (idx = row>>1 fits int16
    for all 50176 rows), with the row parity resolved post-gather by a DVE
    predicated copy.  GPSIMD descriptor generation (~7.9ns/idx) is the
    kernel bottleneck, so total idx count is what matters.
  - L1 table row (384B): [h bf16 x128 | asrc f32 x8 | pad]; gather elem =
    768B pair.  L2 table row (64B): [h2 bf16 x16 | asrc2 f32 | pad];
    gather elem = 512B covering 8 rows, 8-way predicated select.
  - Both layers run in ONE program: L2's table is built per-block from L1
    output (PE transpose + matmul) while L1 gathers still stream, then one
    AllGather and the L2 grid pass.  L2 table order = per-core degree-sorted
    order, so no host roundtrip and no re-permutation.

kernel(**inputs) takes FULL unsharded inputs, returns the FULL [50000, 16]
output.  Host-side numpy does sharding/index prep only; all model math runs
on the NeuronCores.
"""

import os
import sys

import numpy as np

sys.path.insert(0, "/opt/trn_rl_repo")

import concourse.bacc as bacc
import concourse.bass as bass
import concourse.mybir as mybir
import concourse.tile as tile
from concourse.bass_utils import run_bass_kernel_spmd

F32 = mybir.dt.float32
BF16 = mybir.dt.bfloat16
I16 = mybir.dt.int16
U8 = mybir.dt.uint8

N = 50000
NC = 8
OWN = N // NC            # 6250
FIN = 128
HID = 16
HEADS = 8
FH1 = HEADS * HID        # 128
CLS = 16
NEG = 0.2
NPAD = 50176             # 392*128 = 8*6272
NSH = NPAD // NC         # 6272
BLKS = 49
OWNPAD = BLKS * 128      # 6272

ROW1 = 96                # f32 cols: [h bf16 128 (cols 0:64) | asrc 64:72 | pad]
PAIR1 = 2 * ROW1         # 192 f32 = 768B gather elem
ROW2 = 16                # f32 cols: [h2 bf16 16 (cols 0:8) | asrc2 at 8 | pad]
OCT2 = 8 * ROW2          # 128 f32 = 512B gather elem

SC_CAP = 32              # max slot-columns per gather chunk

W1COLS = FH1 + 2 * HEADS     # [W1 | W1@As1 | W1@Ad1] = 144
W2COLS = CLS + 2             # [W2 | W2@As2 | W2@Ad2] = 18


# ---------------------------------------------------------------- host prep

def _prep(edge_index):
    """Grid structures from the edge list (no self-loops). Pure numpy."""
    ei = np.asarray(edge_index)
    src = ei[0].astype(np.int64)
    dst = ei[1].astype(np.int64)

    deg_all = np.zeros((NC, OWNPAD), np.int64)
    per_core = []
    for c in range(NC):
        m = (dst >= c * OWN) & (dst < (c + 1) * OWN)
        s_c = src[m]
        d_c = dst[m] - c * OWN
        deg = np.bincount(d_c, minlength=OWN)
        sigma = np.argsort(-deg, kind="stable")
        rank = np.empty(OWN, np.int64)
        rank[sigma] = np.arange(OWN)
        deg_all[c, :OWN] = deg[sigma]
        per_core.append((s_c, d_c, sigma, rank))

    S = deg_all.reshape(NC, BLKS, 128).max(axis=2).max(axis=0)
    S = np.maximum(S, 2)
    S = S + (S % 2)                       # even for DVE 2x reduce
    MP = np.concatenate([[0], np.cumsum(S)]).astype(int)
    STOT = int(MP[-1])

    # chunks of consecutive blocks with sum(S) <= SC_CAP
    chunks = []
    j = 0
    while j < BLKS:
        k = j + 1
        while k < BLKS and MP[k + 1] - MP[j] <= SC_CAP:
            k += 1
        chunks.append((j, k))
        j = k

    # global rank -> L2 table row (plain: core*6272 + rank)
    cores = []
    for c in range(NC):
        s_c, d_c, sigma, rank = per_core[c]
        r = rank[d_c]
        blk = r // 128
        p = r % 128
        key = blk * 128 + p
        order = np.argsort(key, kind="stable")
        ks = key[order]
        _, fi, fc = np.unique(ks, return_index=True, return_counts=True)
        slot = np.arange(len(ks)) - np.repeat(fi, fc)
        eb, ep_, es = blk[order], p[order], slot
        esrc = s_c[order]
        col = MP[eb] + es

        # L1: table row = padded node id (AllGather layout [core][local])
        row1 = esrc
        idx1 = np.zeros((STOT, 128), np.int16)      # [col, partition]
        par1 = np.zeros((STOT, 128), np.uint8)
        idx1[col, ep_] = (row1 >> 1).astype(np.int16)
        par1[col, ep_] = (row1 & 1).astype(np.uint8)

        # L2: table row = dstcore*6272 + rank_in_dstcore(src)
        cd = esrc // OWN
        r2 = np.empty(len(esrc), np.int64)
        for cc in range(NC):
            mm = cd == cc
            if mm.any():
                r2[mm] = per_core[cc][3][esrc[mm] - cc * OWN]
        row2 = cd * OWNPAD + r2
        idx2 = np.zeros((STOT, 128), np.int16)
        oct2 = np.zeros((STOT, 128), np.int64)
        idx2[col, ep_] = (row2 >> 3).astype(np.int16)
        oct2[col, ep_] = row2 & 7

        pmask = np.zeros((STOT, 128), np.float32)
        pmask[col, ep_] = 1.0
        m8 = np.zeros((8, STOT, 128), np.uint8)
        m8[oct2[col, ep_], col, ep_] = 1.0

        cores.append(dict(
            sigma=sigma,
            idx1=_wrap_idx(idx1.T, chunks, MP),
            idx2=_wrap_idx(idx2.T, chunks, MP),
            pmask=pmask.T.copy(),
            m2o=par1.T.copy(),
            m8=m8.transpose(2, 0, 1).copy(),     # [128, 8, STOT]
        ))

    grids = dict(S=S, MP=MP, STOT=STOT, chunks=chunks)
    return cores, grids


def _wrap_idx(idx_pc, chunks, MP):
    """[128 part, STOT cols] int16 -> per-chunk dma_gather idx layout.

    Within each chunk (cols [MP[j0], MP[j1])), gather position
    i = p + 128*(col-local) is read from sbuf [i%16, i//16], replicated
    across the 8 groups of 16 partitions.  Chunks concatenate to
    [128, 8*STOT].
    """
    P, C = idx_pc.shape
    assert P == 128
    parts = []
    for (j0, j1) in chunks:
        sub = idx_pc[:, MP[j0]:MP[j1]]                 # [128, Sc]
        flat = sub.T.reshape(-1)                       # i = p + 128*c
        t = np.zeros((16, len(flat) // 16), np.int16)
        t[np.arange(len(flat)) % 16, np.arange(len(flat)) // 16] = flat
        parts.append(np.tile(t, (8, 1)))
    return np.concatenate(parts, axis=1)


# ------------------------------------------------------------- bass builder

def _build(grids):
    S, MP, STOT = grids["S"], grids["MP"], grids["STOT"]
    chunks = grids["chunks"]

    nc = bacc.Bacc("TRN2", target_bir_lowering=False, debug=False,
                   num_devices=NC)
    xt = nc.declare_dram_parameter("xt", [128, NSH], BF16, isOutput=False)
    xpermt = nc.declare_dram_parameter("xpermt", [128, OWNPAD], BF16,
                                       isOutput=False)
    w1 = nc.declare_dram_parameter("w1", [128, W1COLS], BF16, isOutput=False)
    w2 = nc.declare_dram_parameter("w2", [128, W2COLS], BF16, isOutput=False)
    b1r = nc.declare_dram_parameter("b1r", [128, FH1], F32, isOutput=False)
    b2r = nc.declare_dram_parameter("b2r", [128, CLS], F32, isOutput=False)
    a2r = nc.declare_dram_parameter("a2r", [128, CLS], F32, isOutput=False)
    ident = nc.declare_dram_parameter("ident", [128, 128], BF16,
                                      isOutput=False)
    idx1 = nc.declare_dram_parameter("idx1", [128, 8 * STOT], I16,
                                     isOutput=False)
    idx2 = nc.declare_dram_parameter("idx2", [128, 8 * STOT], I16,
                                     isOutput=False)
    pmaskp = nc.declare_dram_parameter("pmaskp", [128, STOT], F32,
                                       isOutput=False)
    m2op = nc.declare_dram_parameter("m2op", [128, STOT], U8, isOutput=False)
    m8p = nc.declare_dram_parameter("m8p", [128, 8 * STOT], U8,
                                    isOutput=False)
    out = nc.declare_dram_parameter("out", [OWNPAD, CLS], F32, isOutput=True)
    th1_sh = nc.dram_tensor("th1_sh", [NSH, ROW1], F32)
    th1 = nc.dram_tensor("th1", [NPAD, ROW1], F32, addr_space="Shared")
    th2_sh = nc.dram_tensor("th2_sh", [NSH, ROW2], F32)
    th2 = nc.dram_tensor("th2", [NPAD, ROW2], F32, addr_space="Shared")

    with tile.TileContext(nc) as tc:
        with (
            tc.tile_pool(name="const", bufs=1) as cpool,
            tc.tile_pool(name="xa", bufs=3) as xpool,
            tc.tile_pool(name="stage", bufs=3) as spool,
            tc.tile_pool(name="psA", bufs=2, space="PSUM") as psA,
            tc.tile_pool(name="psB", bufs=2, space="PSUM") as psB,
            tc.tile_pool(name="psT", bufs=2, space="PSUM") as psT,
            tc.tile_pool(name="ps2", bufs=2, space="PSUM") as ps2,
            tc.tile_pool(name="gath", bufs=3) as gpool,
            tc.tile_pool(name="idxp", bufs=3) as ipool,
            tc.tile_pool(name="ep", bufs=2) as epool,
            tc.tile_pool(name="msg", bufs=2) as mpool,
            tc.tile_pool(name="msg3", bufs=3) as m3pool,
            tc.tile_pool(name="fin", bufs=3) as fpool,
            tc.tile_pool(name="blk", bufs=10) as bpool,
        ):
            # resident constants
            w1_sb = cpool.tile([128, W1COLS], BF16)
            nc.sync.dma_start(w1_sb[:], w1[:])
            w2_sb = cpool.tile([128, W2COLS], BF16)
            nc.sync.dma_start(w2_sb[:], w2[:])
            b1_sb = cpool.tile([128, FH1], F32)
            nc.sync.dma_start(b1_sb[:], b1r[:])
            b2_sb = cpool.tile([128, CLS], F32)
            nc.sync.dma_start(b2_sb[:], b2r[:])
            a2r_sb = cpool.tile([128, CLS], F32)
            nc.sync.dma_start(a2r_sb[:], a2r[:])
            id_sb = cpool.tile([128, 128], BF16)
            nc.sync.dma_start(id_sb[:], ident[:])
            pm_sb = cpool.tile([128, STOT], F32)
            nc.sync.dma_start(pm_sb[:], pmaskp[:])
            m2o_sb = cpool.tile([128, STOT], U8)
            nc.sync.dma_start(m2o_sb[:], m2op[:])
            m8_sb = cpool.tile([128, 8, STOT], U8)
            nc.sync.dma_start(m8_sb[:].rearrange("p a b -> p (a b)"), m8p[:])
            # resident per-block L2 self data
            h2own_sb = cpool.tile([128, BLKS, CLS], BF16)
            a2own_sb = cpool.tile([128, BLKS], F32)
            d2own_sb = cpool.tile([128, BLKS], F32)

            # ---- phase A1: th1[n] = [h bf16 | asrc f32 | pad]
            for i7 in range(7):
                xt_t = xpool.tile([128, 896], BF16, tag="xt")
                nc.sync.dma_start(xt_t[:], xt[:, i7 * 896:(i7 + 1) * 896])
                for ii in range(7):
                    i = i7 * 7 + ii
                    ph = psA.tile([128, FH1 + HEADS], F32)
                    nc.tensor.matmul(ph[:], xt_t[:, ii * 128:(ii + 1) * 128],
                                     w1_sb[:, 0:FH1 + HEADS],
                                     start=True, stop=True)
                    st = spool.tile([128, ROW1], F32, tag="st1")
                    nc.scalar.copy(st.bitcast(BF16)[:, 0:FH1], ph[:, 0:FH1])
                    nc.vector.tensor_copy(st[:, 64:72],
                                          ph[:, FH1:FH1 + HEADS])
                    nc.sync.dma_start(th1_sh[i * 128:(i + 1) * 128, :],
                                      st[:])
            tc.strict_bb_all_engine_barrier()
            nc.gpsimd.collective_compute(
                "AllGather", mybir.AluOpType.bypass,
                replica_groups=[list(range(NC))],
                ins=[th1_sh[:]], outs=[th1[:]])
            tc.strict_bb_all_engine_barrier()

            th1v = th1[:].rearrange("(a b) c -> a (b c)", b=2)   # [25088, 192]
            th2v = th2[:].rearrange("(a b) c -> a (b c)", b=8)   # [6272, 128]


            # ---- layer-1 grid pass (+ th2 build per block)
            for ci, (j0, j1) in enumerate(chunks):
                Sc = int(MP[j1] - MP[j0])
                c0 = int(MP[j0])
                i1t = ipool.tile([128, 8 * Sc], I16, tag="i1")
                nc.sync.dma_start(i1t[:], idx1[:, 8 * c0: 8 * (c0 + Sc)])
                g = gpool.tile([128, Sc, PAIR1], F32, tag="g")
                nc.gpsimd.dma_gather(
                    g[:], th1v, i1t[:],
                    num_idxs=128 * Sc, num_idxs_reg=128 * Sc,
                    elem_size=PAIR1, single_packet=False)
                gb = g.bitcast(BF16)    # [128, Sc, 384]
                m2ob = m2o_sb[:, c0:c0 + Sc]

                # per-block own-node matmul -> adst, p_self, h_own
                blkdat = []
                for j in range(j0, j1):
                    xp_t = xpool.tile([128, 128], BF16, tag="xp")
                    nc.sync.dma_start(xp_t[:],
                                      xpermt[:, j * 128:(j + 1) * 128])
                    pb = psB.tile([128, W1COLS], F32, tag="pb")
                    nc.tensor.matmul(pb[:], xp_t[:], w1_sb[:],
                                     start=True, stop=True)
                    adst = bpool.tile([128, HEADS], F32, tag="adst")
                    nc.vector.tensor_copy(adst[:], pb[:, FH1 + HEADS:W1COLS])
                    es = bpool.tile([128, HEADS], F32, tag="es")
                    nc.vector.tensor_tensor(es[:], pb[:, FH1:FH1 + HEADS],
                                            adst[:], op=mybir.AluOpType.add)
                    es2 = bpool.tile([128, HEADS], F32, tag="es2")
                    nc.vector.scalar_tensor_tensor(
                        es2[:], es[:], NEG, es[:],
                        op0=mybir.AluOpType.mult, op1=mybir.AluOpType.max)
                    psf = bpool.tile([128, HEADS], F32, tag="psf")
                    nc.scalar.activation(psf[:], es2[:],
                                         mybir.ActivationFunctionType.Exp)
                    hob = bpool.tile([128, FH1], BF16, tag="hob")
                    nc.scalar.copy(hob[:], pb[:, 0:FH1])
                    blkdat.append((adst, psf, hob))

                # asrc select in [p, h, s] layout (small, strided reads ok)
                ase = epool.tile([128, HEADS, Sc], F32, tag="ase")
                nc.vector.tensor_copy(ase[:],
                                      g[:, :, 64:72].transpose([0, 2, 1]))
                nc.vector.copy_predicated(
                    ase[:],
                    m2ob.unsqueeze(1).broadcast_to([128, HEADS, Sc]),
                    g[:, :, 160:168].transpose([0, 2, 1]))

                # e = lrelu(ase + adst); pm = exp(e) * pmask
                e2 = epool.tile([128, HEADS, Sc], F32, tag="e2")
                for j in range(j0, j1):
                    sl = slice(int(MP[j] - c0), int(MP[j + 1] - c0))
                    nc.vector.tensor_tensor(
                        e2[:, :, sl], ase[:, :, sl],
                        blkdat[j - j0][0][:].unsqueeze(2)
                        .broadcast_to([128, HEADS, int(S[j])]),
                        op=mybir.AluOpType.add)
                nc.vector.scalar_tensor_tensor(
                    e2[:], e2[:], NEG, e2[:],
                    op0=mybir.AluOpType.mult, op1=mybir.AluOpType.max)
                pt = epool.tile([128, HEADS, Sc], F32, tag="pt")
                nc.scalar.activation(pt[:], e2[:],
                                     mybir.ActivationFunctionType.Exp)
                nc.vector.tensor_tensor(
                    pt[:], pt[:],
                    pm_sb[:, c0:c0 + Sc].unsqueeze(1)
                         .broadcast_to([128, HEADS, Sc]),
                    op=mybir.AluOpType.mult)

                # h select (bf16, slot-major contiguous), then ACT transpose
                hsel = mpool.tile([128, Sc, FH1], BF16, tag="hsel")
                nc.vector.tensor_copy(hsel[:], gb[:, :, 0:128])
                nc.vector.copy_predicated(
                    hsel[:],
                    m2ob.unsqueeze(2).broadcast_to([128, Sc, FH1]),
                    gb[:, :, 192:320])
                hst = mpool.tile([128, FH1, Sc], BF16, tag="hst")
                nc.scalar.copy(hst[:], hsel[:].transpose([0, 2, 1]))

                al = epool.tile([128, HEADS, Sc], F32, tag="al")
                for j in range(j0, j1):
                    Sj = int(S[j])
                    sl = slice(int(MP[j] - c0), int(MP[j + 1] - c0))
                    adst, psf, hob = blkdat[j - j0]
                    den = fpool.tile([128, HEADS], F32, tag="den")
                    nc.vector.tensor_reduce(den[:], pt[:, :, sl],
                                            axis=mybir.AxisListType.X,
                                            op=mybir.AluOpType.add)
                    nc.vector.tensor_tensor(den[:], den[:], psf[:],
                                            op=mybir.AluOpType.add)
                    rec = fpool.tile([128, HEADS], F32, tag="rec")
                    nc.vector.reciprocal(rec[:], den[:])
                    nc.vector.tensor_tensor(
                        al[:, :, sl], pt[:, :, sl],
                        rec[:].unsqueeze(2).broadcast_to([128, HEADS, Sj]),
                        op=mybir.AluOpType.mult)
                    nc.vector.tensor_tensor(psf[:], psf[:], rec[:],
                                            op=mybir.AluOpType.mult)

                # msg = hst * alpha, both contiguous in [p, h, c, s]
                msg = m3pool.tile([128, HEADS, HID, Sc], BF16, tag="msg")
                nc.vector.tensor_tensor(
                    msg[:], hst[:].rearrange("p (h c) s -> p h c s", c=HID),
                    al[:].unsqueeze(2).broadcast_to([128, HEADS, HID, Sc]),
                    op=mybir.AluOpType.mult)

                for j in range(j0, j1):
                    sl = slice(int(MP[j] - c0), int(MP[j + 1] - c0))
                    adst, psf, hob = blkdat[j - j0]
                    outun = fpool.tile([128, FH1], F32, tag="outun")
                    mv = msg[:, :, :, sl].rearrange("p h c s -> p (h c) s")
                    nc.vector.tensor_reduce(outun[:], mv,
                                            axis=mybir.AxisListType.X,
                                            op=mybir.AluOpType.add)
                    # + self: h_own * (psf = p_self/den)
                    sm = fpool.tile([128, HEADS, HID], F32, tag="sm")
                    nc.vector.tensor_tensor(
                        sm[:], hob[:].rearrange("p (h c) -> p h c", c=HID),
                        psf[:].unsqueeze(2).broadcast_to([128, HEADS, HID]),
                        op=mybir.AluOpType.mult)
                    nc.vector.tensor_tensor(
                        outun[:], outun[:],
                        sm[:].rearrange("p h c -> p (h c)"),
                        op=mybir.AluOpType.add)
                    nc.vector.tensor_tensor(outun[:], outun[:], b1_sb[:],
                                            op=mybir.AluOpType.add)
                    # x2 = elu(outun) = relu + exp(x - relu) - 1, in bf16
                    r = fpool.tile([128, FH1], F32, tag="r")
                    nc.scalar.activation(r[:], outun[:],
                                         mybir.ActivationFunctionType.Relu)
                    mn = fpool.tile([128, FH1], F32, tag="mn")
                    nc.vector.tensor_tensor(mn[:], outun[:], r[:],
                                            op=mybir.AluOpType.subtract)
                    ex = fpool.tile([128, FH1], F32, tag="ex")
                    nc.scalar.activation(ex[:], mn[:],
                                         mybir.ActivationFunctionType.Exp)
                    finb = fpool.tile([128, FH1], BF16, tag="finb")
                    nc.vector.scalar_tensor_tensor(
                        finb[:], ex[:], -1.0, r[:],
                        op0=mybir.AluOpType.add, op1=mybir.AluOpType.add)

                    # th2 row build: transpose fin, matmul W2ext
                    ptr = psT.tile([128, 128], BF16, tag="ptr")
                    nc.tensor.transpose(ptr[:], finb[:], id_sb[:])
                    ftr = spool.tile([128, 128], BF16, tag="ftr")
                    nc.scalar.copy(ftr[:], ptr[:])
                    p2 = ps2.tile([128, W2COLS], F32, tag="p2")
                    nc.tensor.matmul(p2[:], ftr[:], w2_sb[:],
                                     start=True, stop=True)
                    st2 = spool.tile([128, ROW2 // 2], F32, tag="st2")
                    nc.scalar.copy(st2.bitcast(BF16)[:, 0:CLS], p2[:, 0:CLS])
                    nc.sync.dma_start(th2_sh[j * 128:(j + 1) * 128, 0:8],
                                      st2[:])
                    # keep self data resident
                    nc.scalar.copy(
                        h2own_sb[:, j:j + 1, :].rearrange("p a c -> p (a c)"),
                        p2[:, 0:CLS])
                    nc.vector.tensor_copy(a2own_sb[:, j:j + 1],
                                          p2[:, CLS:CLS + 1])
                    nc.vector.tensor_copy(d2own_sb[:, j:j + 1],
                                          p2[:, CLS + 1:CLS + 2])

            tc.strict_bb_all_engine_barrier()
            nc.gpsimd.collective_compute(
                "AllGather", mybir.AluOpType.bypass,
                replica_groups=[list(range(NC))],
                ins=[th2_sh[:]], outs=[th2[:]])
            tc.strict_bb_all_engine_barrier()

            # ---- layer-2 grid pass
            for (j0, j1) in chunks:
                Sc = int(MP[j1] - MP[j0])
                c0 = int(MP[j0])
                i2t = ipool.tile([128, 8 * Sc], I16, tag="i2")
                nc.sync.dma_start(i2t[:], idx2[:, 8 * c0: 8 * (c0 + Sc)])
                g2 = gpool.tile([128, Sc, OCT2], F32, tag="g")
                nc.gpsimd.dma_gather(
                    g2[:], th2v, i2t[:],
                    num_idxs=128 * Sc, num_idxs_reg=128 * Sc,
                    elem_size=OCT2, single_packet=False)
                g2b = g2.bitcast(BF16)   # [128, Sc, 256]

                # 8-way select of h2 (bf16 cols 32k:32k+16), slot-major
                h2s = mpool.tile([128, Sc, CLS], BF16, tag="hsel")
                nc.vector.tensor_copy(h2s[:], g2b[:, :, 0:16])
                for k in range(1, 8):
                    mk = m8_sb[:, k:k + 1, c0:c0 + Sc].transpose([0, 2, 1])
                    nc.vector.copy_predicated(
                        h2s[:], mk.broadcast_to([128, Sc, CLS]),
                        g2b[:, :, 32 * k:32 * k + 16])
                # asrc2 = h2 . a_src2  (bf16 dot via premult + reduce)
                q2 = mpool.tile([128, Sc, CLS], BF16, tag="q2")
                nc.vector.tensor_tensor(
                    q2[:], h2s[:],
                    a2r_sb[:].unsqueeze(1).broadcast_to([128, Sc, CLS]),
                    op=mybir.AluOpType.mult)
                asr = epool.tile([128, Sc], F32, tag="asr")
                nc.vector.tensor_reduce(asr[:], q2[:],
                                        axis=mybir.AxisListType.X,
                                        op=mybir.AluOpType.add)

                e2 = epool.tile([128, Sc], F32, tag="e2b")
                for j in range(j0, j1):
                    sl = slice(int(MP[j] - c0), int(MP[j + 1] - c0))
                    nc.vector.tensor_tensor(
                        e2[:, sl], asr[:, sl],
                        d2own_sb[:, j:j + 1]
                        .broadcast_to([128, int(S[j])]),
                        op=mybir.AluOpType.add)
                nc.vector.scalar_tensor_tensor(
                    e2[:], e2[:], NEG, e2[:],
                    op0=mybir.AluOpType.mult, op1=mybir.AluOpType.max)
                pt = epool.tile([128, Sc], F32, tag="ptb")
                nc.scalar.activation(pt[:], e2[:],
                                     mybir.ActivationFunctionType.Exp)
                nc.vector.tensor_tensor(pt[:], pt[:], pm_sb[:, c0:c0 + Sc],
                                        op=mybir.AluOpType.mult)

                h2t = mpool.tile([128, CLS, Sc], BF16, tag="hst")
                nc.scalar.copy(h2t[:], h2s[:].transpose([0, 2, 1]))

                al = epool.tile([128, Sc], F32, tag="alb")
                psfs = []
                for j in range(j0, j1):
                    Sj = int(S[j])
                    sl = slice(int(MP[j] - c0), int(MP[j + 1] - c0))
                    es = bpool.tile([128, 1], F32, tag="es")
                    nc.vector.tensor_tensor(es[:], a2own_sb[:, j:j + 1],
                                            d2own_sb[:, j:j + 1],
                                            op=mybir.AluOpType.add)
                    es2 = bpool.tile([128, 1], F32, tag="es2")
                    nc.vector.scalar_tensor_tensor(
                        es2[:], es[:], NEG, es[:],
                        op0=mybir.AluOpType.mult, op1=mybir.AluOpType.max)
                    psf = bpool.tile([128, 1], F32, tag="psf")
                    nc.scalar.activation(psf[:], es2[:],
                                         mybir.ActivationFunctionType.Exp)
                    den = fpool.tile([128, 1], F32, tag="den")
                    nc.vector.tensor_reduce(den[:], pt[:, sl],
                                            axis=mybir.AxisListType.X,
                                            op=mybir.AluOpType.add)
                    nc.vector.tensor_tensor(den[:], den[:], psf[:],
                                            op=mybir.AluOpType.add)
                    rec = fpool.tile([128, 1], F32, tag="rec")
                    nc.vector.reciprocal(rec[:], den[:])
                    nc.vector.tensor_tensor(
                        al[:, sl], pt[:, sl],
                        rec[:].broadcast_to([128, Sj]),
                        op=mybir.AluOpType.mult)
                    nc.vector.tensor_tensor(psf[:], psf[:], rec[:],
                                            op=mybir.AluOpType.mult)
                    psfs.append(psf)

                msg = m3pool.tile([128, CLS, Sc], BF16, tag="msg")
                nc.vector.tensor_tensor(
                    msg[:], h2t[:],
                    al[:].unsqueeze(1).broadcast_to([128, CLS, Sc]),
                    op=mybir.AluOpType.mult)

                for j in range(j0, j1):
                    sl = slice(int(MP[j] - c0), int(MP[j + 1] - c0))
                    psf = psfs[j - j0]
                    o2 = fpool.tile([128, CLS], F32, tag="outun")
                    nc.vector.tensor_reduce(o2[:], msg[:, :, sl],
                                            axis=mybir.AxisListType.X,
                                            op=mybir.AluOpType.add)
                    sm = fpool.tile([128, CLS], F32, tag="sm")
                    nc.vector.tensor_tensor(
                        sm[:],
                        h2own_sb[:, j:j + 1, :].rearrange("p a c -> p (a c)"),
                        psf[:].broadcast_to([128, CLS]),
                        op=mybir.AluOpType.mult)
                    nc.vector.tensor_tensor(o2[:], o2[:], sm[:],
                                            op=mybir.AluOpType.add)
                    fin2 = fpool.tile([128, CLS], F32, tag="fin2")
                    nc.vector.tensor_tensor(fin2[:], o2[:], b2_sb[:],
                                            op=mybir.AluOpType.add)
                    nc.sync.dma_start(out[j * 128:(j + 1) * 128, :], fin2[:])

    nc.compile()
    return nc


# --------------------------------------------------------------- execution

_CACHE = {}
TRACE = os.environ.get("GAT_TRACE", "0") == "1"
RUN_KW = {}


def _to_bf16(a):
    return np.asarray(a, np.float32).astype(mybir.dt.np(BF16))


def _amat(att, fh, hid, heads):
    m = np.zeros((fh, heads), np.float32)
    for h in range(heads):
        m[h * hid:(h + 1) * hid, h] = att[h]
    return m


def kernel(x, edge_index, W1, att_src1, att_dst1, b1, W2, att_src2, att_dst2,
           b2):
    x = np.asarray(x, np.float32)
    ei = np.asarray(edge_index)
    if "prep" not in _CACHE:
        _CACHE["prep"] = _prep(ei)
    cores, grids = _CACHE["prep"]
    if "nc" not in _CACHE:
        _CACHE["nc"] = _build(grids)
    ncb = _CACHE["nc"]

    W1 = np.asarray(W1, np.float32)
    As1 = _amat(np.asarray(att_src1, np.float32), FH1, HID, HEADS)
    Ad1 = _amat(np.asarray(att_dst1, np.float32), FH1, HID, HEADS)
    w1ext = _to_bf16(np.concatenate([W1, W1 @ As1, W1 @ Ad1], axis=1))
    W2 = np.asarray(W2, np.float32)
    As2 = _amat(np.asarray(att_src2, np.float32), CLS, CLS, 1)
    Ad2 = _amat(np.asarray(att_dst2, np.float32), CLS, CLS, 1)
    w2ext = _to_bf16(np.concatenate([W2, W2 @ As2, W2 @ Ad2], axis=1))
    b1row = np.tile(np.asarray(b1, np.float32)[None, :], (128, 1))
    b2row = np.tile(np.asarray(b2, np.float32)[None, :], (128, 1))
    a2row = np.tile(np.asarray(att_src2, np.float32).reshape(1, CLS), (128, 1))
    identity = _to_bf16(np.eye(128, dtype=np.float32))

    xpad = np.zeros((NPAD, FIN), np.float32)
    xpad[:N] = x
    xt = _to_bf16(xpad.T.copy())

    in_maps = []
    for c in range(NC):
        sig = cores[c]["sigma"]
        xperm = np.zeros((OWNPAD, FIN), np.float32)
        xperm[:OWN] = x[c * OWN + sig]
        in_maps.append(dict(
            xt=xt[:, c * NSH:(c + 1) * NSH].copy(),
            xpermt=_to_bf16(xperm.T.copy()),
            w1=w1ext, w2=w2ext, b1r=b1row, b2r=b2row, a2r=a2row,
            ident=identity,
            idx1=cores[c]["idx1"], idx2=cores[c]["idx2"],
            pmaskp=cores[c]["pmask"], m2op=cores[c]["m2o"],
            m8p=cores[c]["m8"].reshape(128, -1).copy(),
        ))
    res = run_bass_kernel_spmd(ncb, in_maps, list(range(NC)),
                               trace=TRACE, **RUN_KW)

    outf = np.zeros((N, CLS), np.float32)
    for c in range(NC):
        sig = cores[c]["sigma"]
        outf[c * OWN + sig] = res.results[c]["out"][:OWN]
    kernel.last_results = (res,)
    return outf


# revision 20
# speedup vs baseline: 1.1666x; 1.1666x over previous
"""GAT (2-layer, PyG-style) on 8 Trainium2 NeuronCores via Bass/Tile.

Strategy (dst-major graph-parallel, gather-descriptor-minimal):
  - Nodes partitioned across 8 cores by dst id (6250 each); per core, own
    dsts are degree-sorted into 49 blocks of 128.  Block j is a
    [128 dst-partitions x S_j slots] grid; slot (d, s) = s-th in-edge of
    block-dst d.  Self-loops are NOT in the grid (computed inline from the
    own-node matmul), so S_j = max plain in-degree -> ~2.5% slot padding.
  - Per-edge source data is fetched with ONE dma_gather per chunk of blocks
    from a DRAM table addressed by PAIRS of rows (idx = row>>1 fits int16
    for all 50176 rows), with the row parity resolved post-gather by a DVE
    predicated copy.  GPSIMD descriptor generation (~7.9ns/idx) is the
    kernel bottleneck, so total idx count is what matters.
  - L1 table row (384B): [h bf16 x128 | asrc f32 x8 | pad]; gather elem =
    768B pair.  L2 table row (64B): [h2 bf16 x16 | asrc2 f32 | pad];
    gather elem = 512B covering 8 rows, 8-way predicated select.
  - Both layers run in ONE program: L2's table is built per-block from L1
    output (PE transpose + matmul) while L1 gathers still stream, then one
    AllGather and the L2 grid pass.  L2 table order = per-core degree-sorted
    order, so no host roundtrip and no re-permutation.

kernel(**inputs) takes FULL unsharded inputs, returns the FULL [50000, 16]
output.  Host-side numpy does sharding/index prep only; all model math runs
on the NeuronCores.
"""

import os
import sys

import numpy as np

sys.path.insert(0, "/opt/trn_rl_repo")

import concourse.bacc as bacc
import concourse.bass as bass
import concourse.mybir as mybir
import concourse.tile as tile
from concourse.bass_utils import run_bass_kernel_spmd

F32 = mybir.dt.float32
BF16 = mybir.dt.bfloat16
I16 = mybir.dt.int16
U8 = mybir.dt.uint8

N = 50000
NC = 8
OWN = N // NC            # 6250
FIN = 128
HID = 16
HEADS = 8
FH1 = HEADS * HID        # 128
CLS = 16
NEG = 0.2
NPAD = 50176             # 392*128 = 8*6272
NSH = NPAD // NC         # 6272
BLKS = 49
OWNPAD = BLKS * 128      # 6272

ROW1 = 64                # f32 cols: [h bf16 x128] only; asrc recomputed on DVE
PAIR1 = 2 * ROW1         # 128 f32 = 512B gather elem
ROW2 = 16                # f32 cols: [h2 bf16 16 (cols 0:8) | asrc2 at 8 | pad]
OCT2 = 8 * ROW2          # 128 f32 = 512B gather elem

SC_CAP = 32              # max slot-columns per gather chunk

W1COLS = FH1 + 2 * HEADS     # [W1 | W1@As1 | W1@Ad1] = 144
W2COLS = CLS + 2             # [W2 | W2@As2 | W2@Ad2] = 18


# ---------------------------------------------------------------- host prep

def _prep(edge_index):
    """Grid structures from the edge list (no self-loops). Pure numpy."""
    ei = np.asarray(edge_index)
    src = ei[0].astype(np.int64)
    dst = ei[1].astype(np.int64)

    deg_all = np.zeros((NC, OWNPAD), np.int64)
    per_core = []
    for c in range(NC):
        m = (dst >= c * OWN) & (dst < (c + 1) * OWN)
        s_c = src[m]
        d_c = dst[m] - c * OWN
        deg = np.bincount(d_c, minlength=OWN)
        sigma = np.argsort(-deg, kind="stable")
        rank = np.empty(OWN, np.int64)
        rank[sigma] = np.arange(OWN)
        deg_all[c, :OWN] = deg[sigma]
        per_core.append((s_c, d_c, sigma, rank))

    S = deg_all.reshape(NC, BLKS, 128).max(axis=2).max(axis=0)
    S = np.maximum(S, 2)
    S = S + (S % 2)                       # even for DVE 2x reduce
    MP = np.concatenate([[0], np.cumsum(S)]).astype(int)
    STOT = int(MP[-1])

    # chunks of consecutive blocks with sum(S) <= SC_CAP
    chunks = []
    j = 0
    while j < BLKS:
        k = j + 1
        while k < BLKS and MP[k + 1] - MP[j] <= SC_CAP:
            k += 1
        chunks.append((j, k))
        j = k

    # global rank -> L2 table row (plain: core*6272 + rank)
    cores = []
    for c in range(NC):
        s_c, d_c, sigma, rank = per_core[c]
        r = rank[d_c]
        blk = r // 128
        p = r % 128
        key = blk * 128 + p
        order = np.argsort(key, kind="stable")
        ks = key[order]
        _, fi, fc = np.unique(ks, return_index=True, return_counts=True)
        slot = np.arange(len(ks)) - np.repeat(fi, fc)
        eb, ep_, es = blk[order], p[order], slot
        esrc = s_c[order]
        col = MP[eb] + es

        # L1: table row = padded node id (AllGather layout [core][local])
        row1 = esrc
        idx1 = np.zeros((STOT, 128), np.int16)      # [col, partition]
        par1 = np.zeros((STOT, 128), np.uint8)
        idx1[col, ep_] = (row1 >> 1).astype(np.int16)
        par1[col, ep_] = (row1 & 1).astype(np.uint8)

        # L2: table row = dstcore*6272 + rank_in_dstcore(src)
        cd = esrc // OWN
        r2 = np.empty(len(esrc), np.int64)
        for cc in range(NC):
            mm = cd == cc
            if mm.any():
                r2[mm] = per_core[cc][3][esrc[mm] - cc * OWN]
        row2 = cd * OWNPAD + r2
        idx2 = np.zeros((STOT, 128), np.int16)
        oct2 = np.zeros((STOT, 128), np.int64)
        idx2[col, ep_] = (row2 >> 3).astype(np.int16)
        oct2[col, ep_] = row2 & 7

        pmask = np.zeros((STOT, 128), np.float32)
        pmask[col, ep_] = 1.0
        m8 = np.zeros((8, STOT, 128), np.uint8)
        m8[oct2[col, ep_], col, ep_] = 1.0

        cores.append(dict(
            sigma=sigma,
            idx1=_wrap_idx(idx1.T, chunks, MP),
            idx2=_wrap_idx(idx2.T, chunks, MP),
            pmask=pmask.T.copy(),
            m2o=par1.T.copy(),
            m8=m8.transpose(2, 0, 1).copy(),     # [128, 8, STOT]
        ))

    grids = dict(S=S, MP=MP, STOT=STOT, chunks=chunks)
    return cores, grids


def _wrap_idx(idx_pc, chunks, MP):
    """[128 part, STOT cols] int16 -> per-chunk dma_gather idx layout.

    Within each chunk (cols [MP[j0], MP[j1])), gather position
    i = p + 128*(col-local) is read from sbuf [i%16, i//16], replicated
    across the 8 groups of 16 partitions.  Chunks concatenate to
    [128, 8*STOT].
    """
    P, C = idx_pc.shape
    assert P == 128
    parts = []
    for (j0, j1) in chunks:
        sub = idx_pc[:, MP[j0]:MP[j1]]                 # [128, Sc]
        flat = sub.T.reshape(-1)                       # i = p + 128*c
        t = np.zeros((16, len(flat) // 16), np.int16)
        t[np.arange(len(flat)) % 16, np.arange(len(flat)) // 16] = flat
        parts.append(np.tile(t, (8, 1)))
    return np.concatenate(parts, axis=1)


# ------------------------------------------------------------- bass builder

def _build(grids):
    S, MP, STOT = grids["S"], grids["MP"], grids["STOT"]
    chunks = grids["chunks"]

    nc = bacc.Bacc("TRN2", target_bir_lowering=False, debug=False,
                   num_devices=NC)
    xt = nc.declare_dram_parameter("xt", [128, NSH], BF16, isOutput=False)
    xpermt = nc.declare_dram_parameter("xpermt", [128, OWNPAD], BF16,
                                       isOutput=False)
    w1 = nc.declare_dram_parameter("w1", [128, W1COLS], BF16, isOutput=False)
    w2 = nc.declare_dram_parameter("w2", [128, W2COLS], BF16, isOutput=False)
    b1r = nc.declare_dram_parameter("b1r", [128, FH1], F32, isOutput=False)
    b2r = nc.declare_dram_parameter("b2r", [128, CLS], F32, isOutput=False)
    a2r = nc.declare_dram_parameter("a2r", [128, CLS], F32, isOutput=False)
    a1r = nc.declare_dram_parameter("a1r", [128, FH1], F32, isOutput=False)
    ident = nc.declare_dram_parameter("ident", [128, 128], BF16,
                                      isOutput=False)
    idx1 = nc.declare_dram_parameter("idx1", [128, 8 * STOT], I16,
                                     isOutput=False)
    idx2 = nc.declare_dram_parameter("idx2", [128, 8 * STOT], I16,
                                     isOutput=False)
    pmaskp = nc.declare_dram_parameter("pmaskp", [128, STOT], F32,
                                       isOutput=False)
    m2op = nc.declare_dram_parameter("m2op", [128, STOT], U8, isOutput=False)
    m8p = nc.declare_dram_parameter("m8p", [128, 8 * STOT], U8,
                                    isOutput=False)
    out = nc.declare_dram_parameter("out", [OWNPAD, CLS], F32, isOutput=True)
    th1_sh = nc.dram_tensor("th1_sh", [NSH, ROW1], F32)
    th1 = nc.dram_tensor("th1", [NPAD, ROW1], F32, addr_space="Shared")
    th2_sh = nc.dram_tensor("th2_sh", [NSH, ROW2], F32)
    th2 = nc.dram_tensor("th2", [NPAD, ROW2], F32, addr_space="Shared")

    with tile.TileContext(nc) as tc:
        with (
            tc.tile_pool(name="const", bufs=1) as cpool,
            tc.tile_pool(name="xa", bufs=3) as xpool,
            tc.tile_pool(name="stage", bufs=3) as spool,
            tc.tile_pool(name="psA", bufs=2, space="PSUM") as psA,
            tc.tile_pool(name="psB", bufs=2, space="PSUM") as psB,
            tc.tile_pool(name="psT", bufs=2, space="PSUM") as psT,
            tc.tile_pool(name="ps2", bufs=2, space="PSUM") as ps2,
            tc.tile_pool(name="gath", bufs=3) as gpool,
            tc.tile_pool(name="idxp", bufs=3) as ipool,
            tc.tile_pool(name="ep", bufs=2) as epool,
            tc.tile_pool(name="msg", bufs=2) as mpool,
            tc.tile_pool(name="msg3", bufs=3) as m3pool,
            tc.tile_pool(name="fin", bufs=3) as fpool,
            tc.tile_pool(name="blk", bufs=10) as bpool,
        ):
            # resident constants
            w1_sb = cpool.tile([128, W1COLS], BF16)
            nc.sync.dma_start(w1_sb[:], w1[:])
            w2_sb = cpool.tile([128, W2COLS], BF16)
            nc.sync.dma_start(w2_sb[:], w2[:])
            b1_sb = cpool.tile([128, FH1], F32)
            nc.sync.dma_start(b1_sb[:], b1r[:])
            b2_sb = cpool.tile([128, CLS], F32)
            nc.sync.dma_start(b2_sb[:], b2r[:])
            a2r_sb = cpool.tile([128, CLS], F32)
            nc.sync.dma_start(a2r_sb[:], a2r[:])
            a1r_sb = cpool.tile([128, FH1], F32)
            nc.sync.dma_start(a1r_sb[:], a1r[:])
            id_sb = cpool.tile([128, 128], BF16)
            nc.sync.dma_start(id_sb[:], ident[:])
            pm_sb = cpool.tile([128, STOT], F32)
            nc.sync.dma_start(pm_sb[:], pmaskp[:])
            m2o_sb = cpool.tile([128, STOT], U8)
            nc.sync.dma_start(m2o_sb[:], m2op[:])
            m8_sb = cpool.tile([128, 8, STOT], U8)
            nc.sync.dma_start(m8_sb[:].rearrange("p a b -> p (a b)"), m8p[:])
            # resident per-block L2 self data
            h2own_sb = cpool.tile([128, BLKS, CLS], BF16)
            a2own_sb = cpool.tile([128, BLKS], F32)
            d2own_sb = cpool.tile([128, BLKS], F32)

            # ---- phase A1: th1[n] = [h bf16 | asrc f32 | pad]
            for i7 in range(7):
                xt_t = xpool.tile([128, 896], BF16, tag="xt")
                nc.sync.dma_start(xt_t[:], xt[:, i7 * 896:(i7 + 1) * 896])
                for ii in range(7):
                    i = i7 * 7 + ii
                    ph = psA.tile([128, FH1], F32)
                    nc.tensor.matmul(ph[:], xt_t[:, ii * 128:(ii + 1) * 128],
                                     w1_sb[:, 0:FH1],
                                     start=True, stop=True)
                    st = spool.tile([128, ROW1], F32, tag="st1")
                    nc.scalar.copy(st.bitcast(BF16)[:, 0:FH1], ph[:, 0:FH1])
                    nc.sync.dma_start(th1_sh[i * 128:(i + 1) * 128, :],
                                      st[:])
            tc.strict_bb_all_engine_barrier()
            nc.gpsimd.collective_compute(
                "AllGather", mybir.AluOpType.bypass,
                replica_groups=[list(range(NC))],
                ins=[th1_sh[:]], outs=[th1[:]])
            tc.strict_bb_all_engine_barrier()

            th1v = th1[:].rearrange("(a b) c -> a (b c)", b=2)   # [25088, 192]
            th2v = th2[:].rearrange("(a b) c -> a (b c)", b=8)   # [6272, 128]


            # ---- layer-1 grid pass (+ th2 build per block)
            for ci, (j0, j1) in enumerate(chunks):
                Sc = int(MP[j1] - MP[j0])
                c0 = int(MP[j0])
                i1t = ipool.tile([128, 8 * Sc], I16, tag="i1")
                nc.sync.dma_start(i1t[:], idx1[:, 8 * c0: 8 * (c0 + Sc)])
                g = gpool.tile([128, Sc, PAIR1], F32, tag="g")
                nc.gpsimd.dma_gather(
                    g[:], th1v, i1t[:],
                    num_idxs=128 * Sc, num_idxs_reg=128 * Sc,
                    elem_size=PAIR1, single_packet=False)
                gb = g.bitcast(BF16)    # [128, Sc, 256]
                m2ob = m2o_sb[:, c0:c0 + Sc]

                # h select (bf16, slot-major contiguous), then ACT transpose
                hsel = mpool.tile([128, Sc, FH1], BF16, tag="hsel")
                nc.vector.tensor_copy(hsel[:], gb[:, :, 0:128])
                nc.vector.copy_predicated(
                    hsel[:],
                    m2ob.unsqueeze(2).broadcast_to([128, Sc, FH1]),
                    gb[:, :, 128:256])
                hst = mpool.tile([128, FH1, Sc], BF16, tag="hst")
                nc.scalar.copy(hst[:], hsel[:].transpose([0, 2, 1]))

                # asrc = per-head dot(h, a_src): contiguous mult then
                # in-place tree adds (bf16), final add to f32
                q = epool.tile([128, FH1, Sc], BF16, tag="q")
                nc.vector.tensor_tensor(
                    q[:], hst[:],
                    a1r_sb[:].unsqueeze(2).broadcast_to([128, FH1, Sc]),
                    op=mybir.AluOpType.mult)
                qv = q[:].rearrange("p (h c) s -> p h c s", c=HID)
                nc.vector.tensor_tensor(qv[:, :, 0:8, :], qv[:, :, 0:8, :],
                                        qv[:, :, 8:16, :],
                                        op=mybir.AluOpType.add)
                nc.vector.tensor_tensor(qv[:, :, 0:4, :], qv[:, :, 0:4, :],
                                        qv[:, :, 4:8, :],
                                        op=mybir.AluOpType.add)
                nc.vector.tensor_tensor(qv[:, :, 0:2, :], qv[:, :, 0:2, :],
                                        qv[:, :, 2:4, :],
                                        op=mybir.AluOpType.add)
                ase = epool.tile([128, HEADS, Sc], F32, tag="ase")
                nc.vector.tensor_tensor(ase[:].unsqueeze(2),
                                        qv[:, :, 0:1, :], qv[:, :, 1:2, :],
                                        op=mybir.AluOpType.add)

                # per-block own-node matmul -> adst, p_self, h_own
                blkdat = []
                for j in range(j0, j1):
                    xp_t = xpool.tile([128, 128], BF16, tag="xp")
                    nc.sync.dma_start(xp_t[:],
                                      xpermt[:, j * 128:(j + 1) * 128])
                    pb = psB.tile([128, W1COLS], F32, tag="pb")
                    nc.tensor.matmul(pb[:], xp_t[:], w1_sb[:],
                                     start=True, stop=True)
                    adst = bpool.tile([128, HEADS], F32, tag="adst")
                    nc.vector.tensor_copy(adst[:], pb[:, FH1 + HEADS:W1COLS])
                    es = bpool.tile([128, HEADS], F32, tag="es")
                    nc.vector.tensor_tensor(es[:], pb[:, FH1:FH1 + HEADS],
                                            adst[:], op=mybir.AluOpType.add)
                    es2 = bpool.tile([128, HEADS], F32, tag="es2")
                    nc.vector.scalar_tensor_tensor(
                        es2[:], es[:], NEG, es[:],
                        op0=mybir.AluOpType.mult, op1=mybir.AluOpType.max)
                    psf = bpool.tile([128, HEADS], F32, tag="psf")
                    nc.scalar.activation(psf[:], es2[:],
                                         mybir.ActivationFunctionType.Exp)
                    hob = bpool.tile([128, FH1], BF16, tag="hob")
                    nc.scalar.copy(hob[:], pb[:, 0:FH1])
                    blkdat.append((adst, psf, hob))

                # e = lrelu(ase + adst); pm = exp(e) * pmask
                e2 = epool.tile([128, HEADS, Sc], F32, tag="e2")
                for j in range(j0, j1):
                    sl = slice(int(MP[j] - c0), int(MP[j + 1] - c0))
                    nc.vector.tensor_tensor(
                        e2[:, :, sl], ase[:, :, sl],
                        blkdat[j - j0][0][:].unsqueeze(2)
                        .broadcast_to([128, HEADS, int(S[j])]),
                        op=mybir.AluOpType.add)
                nc.vector.scalar_tensor_tensor(
                    e2[:], e2[:], NEG, e2[:],
                    op0=mybir.AluOpType.mult, op1=mybir.AluOpType.max)
                pt = epool.tile([128, HEADS, Sc], F32, tag="pt")
                nc.scalar.activation(pt[:], e2[:],
                                     mybir.ActivationFunctionType.Exp)
                nc.vector.tensor_tensor(
                    pt[:], pt[:],
                    pm_sb[:, c0:c0 + Sc].unsqueeze(1)
                         .broadcast_to([128, HEADS, Sc]),
                    op=mybir.AluOpType.mult)

                al = epool.tile([128, HEADS, Sc], F32, tag="al")
                for j in range(j0, j1):
                    Sj = int(S[j])
                    sl = slice(int(MP[j] - c0), int(MP[j + 1] - c0))
                    adst, psf, hob = blkdat[j - j0]
                    den = fpool.tile([128, HEADS], F32, tag="den")
                    nc.vector.tensor_reduce(den[:], pt[:, :, sl],
                                            axis=mybir.AxisListType.X,
                                            op=mybir.AluOpType.add)
                    nc.vector.tensor_tensor(den[:], den[:], psf[:],
                                            op=mybir.AluOpType.add)
                    rec = fpool.tile([128, HEADS], F32, tag="rec")
                    nc.vector.reciprocal(rec[:], den[:])
                    nc.vector.tensor_tensor(
                        al[:, :, sl], pt[:, :, sl],
                        rec[:].unsqueeze(2).broadcast_to([128, HEADS, Sj]),
                        op=mybir.AluOpType.mult)
                    nc.vector.tensor_tensor(psf[:], psf[:], rec[:],
                                            op=mybir.AluOpType.mult)

                # msg = hst * alpha, both contiguous in [p, h, c, s]
                msg = m3pool.tile([128, HEADS, HID, Sc], BF16, tag="msg")
                nc.vector.tensor_tensor(
                    msg[:], hst[:].rearrange("p (h c) s -> p h c s", c=HID),
                    al[:].unsqueeze(2).broadcast_to([128, HEADS, HID, Sc]),
                    op=mybir.AluOpType.mult)

                for j in range(j0, j1):
                    sl = slice(int(MP[j] - c0), int(MP[j + 1] - c0))
                    adst, psf, hob = blkdat[j - j0]
                    outun = fpool.tile([128, FH1], F32, tag="outun")
                    mv = msg[:, :, :, sl].rearrange("p h c s -> p (h c) s")
                    nc.vector.tensor_reduce(outun[:], mv,
                                            axis=mybir.AxisListType.X,
                                            op=mybir.AluOpType.add)
                    # + self: h_own * (psf = p_self/den)
                    sm = fpool.tile([128, HEADS, HID], F32, tag="sm")
                    nc.vector.tensor_tensor(
                        sm[:], hob[:].rearrange("p (h c) -> p h c", c=HID),
                        psf[:].unsqueeze(2).broadcast_to([128, HEADS, HID]),
                        op=mybir.AluOpType.mult)
                    nc.vector.tensor_tensor(
                        outun[:], outun[:],
                        sm[:].rearrange("p h c -> p (h c)"),
                        op=mybir.AluOpType.add)
                    nc.vector.tensor_tensor(outun[:], outun[:], b1_sb[:],
                                            op=mybir.AluOpType.add)
                    # x2 = elu(outun) = relu + exp(x - relu) - 1, in bf16
                    r = fpool.tile([128, FH1], F32, tag="r")
                    nc.scalar.activation(r[:], outun[:],
                                         mybir.ActivationFunctionType.Relu)
                    mn = fpool.tile([128, FH1], F32, tag="mn")
                    nc.vector.tensor_tensor(mn[:], outun[:], r[:],
                                            op=mybir.AluOpType.subtract)
                    ex = fpool.tile([128, FH1], F32, tag="ex")
                    nc.scalar.activation(ex[:], mn[:],
                                         mybir.ActivationFunctionType.Exp)
                    finb = fpool.tile([128, FH1], BF16, tag="finb")
                    nc.vector.scalar_tensor_tensor(
                        finb[:], ex[:], -1.0, r[:],
                        op0=mybir.AluOpType.add, op1=mybir.AluOpType.add)

                    # th2 row build: transpose fin, matmul W2ext
                    ptr = psT.tile([128, 128], BF16, tag="ptr")
                    nc.tensor.transpose(ptr[:], finb[:], id_sb[:])
                    ftr = spool.tile([128, 128], BF16, tag="ftr")
                    nc.scalar.copy(ftr[:], ptr[:])
                    p2 = ps2.tile([128, W2COLS], F32, tag="p2")
                    nc.tensor.matmul(p2[:], ftr[:], w2_sb[:],
                                     start=True, stop=True)
                    st2 = spool.tile([128, ROW2 // 2], F32, tag="st2")
                    nc.scalar.copy(st2.bitcast(BF16)[:, 0:CLS], p2[:, 0:CLS])
                    nc.sync.dma_start(th2_sh[j * 128:(j + 1) * 128, 0:8],
                                      st2[:])
                    # keep self data resident
                    nc.scalar.copy(
                        h2own_sb[:, j:j + 1, :].rearrange("p a c -> p (a c)"),
                        p2[:, 0:CLS])
                    nc.vector.tensor_copy(a2own_sb[:, j:j + 1],
                                          p2[:, CLS:CLS + 1])
                    nc.vector.tensor_copy(d2own_sb[:, j:j + 1],
                                          p2[:, CLS + 1:CLS + 2])

            tc.strict_bb_all_engine_barrier()
            nc.gpsimd.collective_compute(
                "AllGather", mybir.AluOpType.bypass,
                replica_groups=[list(range(NC))],
                ins=[th2_sh[:]], outs=[th2[:]])
            tc.strict_bb_all_engine_barrier()

            # ---- layer-2 grid pass
            for (j0, j1) in chunks:
                Sc = int(MP[j1] - MP[j0])
                c0 = int(MP[j0])
                i2t = ipool.tile([128, 8 * Sc], I16, tag="i2")
                nc.sync.dma_start(i2t[:], idx2[:, 8 * c0: 8 * (c0 + Sc)])
                g2 = gpool.tile([128, Sc, OCT2], F32, tag="g")
                nc.gpsimd.dma_gather(
                    g2[:], th2v, i2t[:],
                    num_idxs=128 * Sc, num_idxs_reg=128 * Sc,
                    elem_size=OCT2, single_packet=False)
                g2b = g2.bitcast(BF16)   # [128, Sc, 256]

                # 8-way select of h2 (bf16 cols 32k:32k+16), slot-major
                h2s = mpool.tile([128, Sc, CLS], BF16, tag="hsel")
                nc.vector.tensor_copy(h2s[:], g2b[:, :, 0:16])
                for k in range(1, 8):
                    mk = m8_sb[:, k:k + 1, c0:c0 + Sc].transpose([0, 2, 1])
                    nc.vector.copy_predicated(
                        h2s[:], mk.broadcast_to([128, Sc, CLS]),
                        g2b[:, :, 32 * k:32 * k + 16])
                # asrc2 = h2 . a_src2  (bf16 dot via premult + reduce)
                q2 = mpool.tile([128, Sc, CLS], BF16, tag="q2")
                nc.vector.tensor_tensor(
                    q2[:], h2s[:],
                    a2r_sb[:].unsqueeze(1).broadcast_to([128, Sc, CLS]),
                    op=mybir.AluOpType.mult)
                asr = epool.tile([128, Sc], F32, tag="asr")
                nc.vector.tensor_reduce(asr[:], q2[:],
                                        axis=mybir.AxisListType.X,
                                        op=mybir.AluOpType.add)

                e2 = epool.tile([128, Sc], F32, tag="e2b")
                for j in range(j0, j1):
                    sl = slice(int(MP[j] - c0), int(MP[j + 1] - c0))
                    nc.vector.tensor_tensor(
                        e2[:, sl], asr[:, sl],
                        d2own_sb[:, j:j + 1]
                        .broadcast_to([128, int(S[j])]),
                        op=mybir.AluOpType.add)
                nc.vector.scalar_tensor_tensor(
                    e2[:], e2[:], NEG, e2[:],
                    op0=mybir.AluOpType.mult, op1=mybir.AluOpType.max)
                pt = epool.tile([128, Sc], F32, tag="ptb")
                nc.scalar.activation(pt[:], e2[:],
                                     mybir.ActivationFunctionType.Exp)
                nc.vector.tensor_tensor(pt[:], pt[:], pm_sb[:, c0:c0 + Sc],
                                        op=mybir.AluOpType.mult)

                h2t = mpool.tile([128, CLS, Sc], BF16, tag="hst")
                nc.scalar.copy(h2t[:], h2s[:].transpose([0, 2, 1]))

                al = epool.tile([128, Sc], F32, tag="alb")
                psfs = []
                for j in range(j0, j1):
                    Sj = int(S[j])
                    sl = slice(int(MP[j] - c0), int(MP[j + 1] - c0))
                    es = bpool.tile([128, 1], F32, tag="es")
                    nc.vector.tensor_tensor(es[:], a2own_sb[:, j:j + 1],
                                            d2own_sb[:, j:j + 1],
                                            op=mybir.AluOpType.add)
                    es2 = bpool.tile([128, 1], F32, tag="es2")
                    nc.vector.scalar_tensor_tensor(
                        es2[:], es[:], NEG, es[:],
                        op0=mybir.AluOpType.mult, op1=mybir.AluOpType.max)
                    psf = bpool.tile([128, 1], F32, tag="psf")
                    nc.scalar.activation(psf[:], es2[:],
                                         mybir.ActivationFunctionType.Exp)
                    den = fpool.tile([128, 1], F32, tag="den")
                    nc.vector.tensor_reduce(den[:], pt[:, sl],
                                            axis=mybir.AxisListType.X,
                                            op=mybir.AluOpType.add)
                    nc.vector.tensor_tensor(den[:], den[:], psf[:],
                                            op=mybir.AluOpType.add)
                    rec = fpool.tile([128, 1], F32, tag="rec")
                    nc.vector.reciprocal(rec[:], den[:])
                    nc.vector.tensor_tensor(
                        al[:, sl], pt[:, sl],
                        rec[:].broadcast_to([128, Sj]),
                        op=mybir.AluOpType.mult)
                    nc.vector.tensor_tensor(psf[:], psf[:], rec[:],
                                            op=mybir.AluOpType.mult)
                    psfs.append(psf)

                msg = m3pool.tile([128, CLS, Sc], BF16, tag="msg")
                nc.vector.tensor_tensor(
                    msg[:], h2t[:],
                    al[:].unsqueeze(1).broadcast_to([128, CLS, Sc]),
                    op=mybir.AluOpType.mult)

                for j in range(j0, j1):
                    sl = slice(int(MP[j] - c0), int(MP[j + 1] - c0))
                    psf = psfs[j - j0]
                    o2 = fpool.tile([128, CLS], F32, tag="outun")
                    nc.vector.tensor_reduce(o2[:], msg[:, :, sl],
                                            axis=mybir.AxisListType.X,
                                            op=mybir.AluOpType.add)
                    sm = fpool.tile([128, CLS], F32, tag="sm")
                    nc.vector.tensor_tensor(
                        sm[:],
                        h2own_sb[:, j:j + 1, :].rearrange("p a c -> p (a c)"),
                        psf[:].broadcast_to([128, CLS]),
                        op=mybir.AluOpType.mult)
                    nc.vector.tensor_tensor(o2[:], o2[:], sm[:],
                                            op=mybir.AluOpType.add)
                    fin2 = fpool.tile([128, CLS], F32, tag="fin2")
                    nc.vector.tensor_tensor(fin2[:], o2[:], b2_sb[:],
                                            op=mybir.AluOpType.add)
                    nc.sync.dma_start(out[j * 128:(j + 1) * 128, :], fin2[:])

    nc.compile()
    return nc


# --------------------------------------------------------------- execution

_CACHE = {}
TRACE = os.environ.get("GAT_TRACE", "0") == "1"
RUN_KW = {}


def _to_bf16(a):
    return np.asarray(a, np.float32).astype(mybir.dt.np(BF16))


def _amat(att, fh, hid, heads):
    m = np.zeros((fh, heads), np.float32)
    for h in range(heads):
        m[h * hid:(h + 1) * hid, h] = att[h]
    return m


def kernel(x, edge_index, W1, att_src1, att_dst1, b1, W2, att_src2, att_dst2,
           b2):
    x = np.asarray(x, np.float32)
    ei = np.asarray(edge_index)
    if "prep" not in _CACHE:
        _CACHE["prep"] = _prep(ei)
    cores, grids = _CACHE["prep"]
    if "nc" not in _CACHE:
        _CACHE["nc"] = _build(grids)
    ncb = _CACHE["nc"]

    W1 = np.asarray(W1, np.float32)
    As1 = _amat(np.asarray(att_src1, np.float32), FH1, HID, HEADS)
    Ad1 = _amat(np.asarray(att_dst1, np.float32), FH1, HID, HEADS)
    w1ext = _to_bf16(np.concatenate([W1, W1 @ As1, W1 @ Ad1], axis=1))
    W2 = np.asarray(W2, np.float32)
    As2 = _amat(np.asarray(att_src2, np.float32), CLS, CLS, 1)
    Ad2 = _amat(np.asarray(att_dst2, np.float32), CLS, CLS, 1)
    w2ext = _to_bf16(np.concatenate([W2, W2 @ As2, W2 @ Ad2], axis=1))
    b1row = np.tile(np.asarray(b1, np.float32)[None, :], (128, 1))
    b2row = np.tile(np.asarray(b2, np.float32)[None, :], (128, 1))
    a2row = np.tile(np.asarray(att_src2, np.float32).reshape(1, CLS), (128, 1))
    a1row = np.tile(np.asarray(att_src1, np.float32).reshape(1, FH1), (128, 1))
    identity = _to_bf16(np.eye(128, dtype=np.float32))

    xpad = np.zeros((NPAD, FIN), np.float32)
    xpad[:N] = x
    xt = _to_bf16(xpad.T.copy())

    in_maps = []
    for c in range(NC):
        sig = cores[c]["sigma"]
        xperm = np.zeros((OWNPAD, FIN), np.float32)
        xperm[:OWN] = x[c * OWN + sig]
        in_maps.append(dict(
            xt=xt[:, c * NSH:(c + 1) * NSH].copy(),
            xpermt=_to_bf16(xperm.T.copy()),
            w1=w1ext, w2=w2ext, b1r=b1row, b2r=b2row, a2r=a2row, a1r=a1row,
            ident=identity,
            idx1=cores[c]["idx1"], idx2=cores[c]["idx2"],
            pmaskp=cores[c]["pmask"], m2op=cores[c]["m2o"],
            m8p=cores[c]["m8"].reshape(128, -1).copy(),
        ))
    res = run_bass_kernel_spmd(ncb, in_maps, list(range(NC)),
                               trace=TRACE, **RUN_KW)

    outf = np.zeros((N, CLS), np.float32)
    for c in range(NC):
        sig = cores[c]["sigma"]
        outf[c * OWN + sig] = res.results[c]["out"][:OWN]
    kernel.last_results = (res,)
    return outf
